# revision 8
# baseline (speedup 1.0000x reference)
"""Trainium2 Bass kernel for nn_DenseFilterExpansion.

Computes out[b, f, t] = x[b, 0, t] * w[f, t] + bias[f, t] for
x: (128, 1, 4096), w/bias: (256, 4096)  ->  out: (128, 256, 4096) fp32.

Strategy (per core, data-parallel over batch, 16 batches/core):
  - w (and bias, if nonzero) stay resident in SBUF (4 MB each).
  - For each batch b: TensorE broadcasts the row x[b, :] across 128
    partitions via a K=1 ones-matmul into PSUM (nearly free).
  - VectorE multiplies the resident w chunk (128 filters x 4096) by the
    PSUM broadcast, writing (128, bg, 4096) SBUF out tiles.
  - Large contiguous DMAs (HWDGE) store each tile; per partition the
    output row is 16 KB contiguous in DRAM.
The kernel is HBM-write-bound (64 MB of output per core).
"""

import numpy as np

import concourse.bacc as bacc
import concourse.bass as bass
import concourse.mybir as mybir
import concourse.tile as tile
from concourse.bass_utils import run_bass_kernel_spmd

N_CORES = 8
B_FULL = 128
F = 256
T = 4096
BS = B_FULL // N_CORES  # batches per core = 16
P = 128                 # partitions
FP = F // P             # f-chunks = 2
BG = 2                  # batches per output tile
NG = BS // BG           # output tile groups = 8
TH = 2048               # half of T (psum tile width, 4 banks)
MM_N = 512              # matmul free dim (one PSUM bank)

_nc_cache: dict = {}


def _build(with_bias: bool) -> bass.Bass:
    f32 = mybir.dt.float32
    nc = bacc.Bacc("TRN2", debug=False)

    x_d = nc.dram_tensor("x", [BS, T], f32, kind="ExternalInput")
    w_d = nc.dram_tensor("w", [F, T], f32, kind="ExternalInput")
    b_d = (
        nc.dram_tensor("bvec", [F, T], f32, kind="ExternalInput")
        if with_bias
        else None
    )
    o_d = nc.dram_tensor("out", [BS, F, T], f32, kind="ExternalOutput")

    out_bufs = 2 if with_bias else 3
    with tile.TileContext(nc) as tc:
        with (
            tc.tile_pool(name="const", bufs=1) as cpool,
            tc.tile_pool(name="xstage", bufs=3) as xpool,
            tc.tile_pool(name="outp", bufs=out_bufs) as opool,
            tc.tile_pool(name="psum", bufs=2, space="PSUM") as ppool,
        ):
            ones = cpool.tile([1, P], f32, tag="ones")
            nc.vector.memset(ones[:], 1.0)

            w_sb = []
            b_sb = []
            for c in range(FP):
                wt = cpool.tile([P, T], f32, tag=f"w{c}")
                nc.sync.dma_start(out=wt[:], in_=w_d[c * P : (c + 1) * P, :])
                w_sb.append(wt)
                if with_bias:
                    bt = cpool.tile([P, T], f32, tag=f"b{c}")
                    nc.sync.dma_start(out=bt[:], in_=b_d[c * P : (c + 1) * P, :])
                    b_sb.append(bt)

            for g in range(NG):
                otiles = [
                    opool.tile([P, BG, T], f32, tag="otile", name=f"ot{g}_{c}")
                    for c in range(FP)
                ]
                for bl in range(BG):
                    bi = g * BG + bl
                    x_row = xpool.tile([1, T], f32, tag="xrow", name=f"xr{bi}")
                    nc.sync.dma_start(out=x_row[:], in_=x_d[bi : bi + 1, :])
                    for h in range(T // TH):
                        ps = ppool.tile([P, TH], f32, tag="ps", name=f"ps{bi}_{h}")
                        for j in range(TH // MM_N):
                            col = h * TH + j * MM_N
                            nc.tensor.matmul(
                                ps[:, j * MM_N : (j + 1) * MM_N],
                                ones[:],
                                x_row[0:1, col : col + MM_N],
                                start=True,
                                stop=True,
                            )
                        for c in range(FP):
                            nc.vector.tensor_mul(
                                out=otiles[c][:, bl, h * TH : (h + 1) * TH],
                                in0=w_sb[c][:, h * TH : (h + 1) * TH],
                                in1=ps[:],
                            )
                            if with_bias:
                                nc.vector.tensor_add(
                                    out=otiles[c][:, bl, h * TH : (h + 1) * TH],
                                    in0=otiles[c][:, bl, h * TH : (h + 1) * TH],
                                    in1=b_sb[c][:, h * TH : (h + 1) * TH],
                                )
                for c in range(FP):
                    nc.sync.dma_start(
                        out=o_d[
                            g * BG : (g + 1) * BG, c * P : (c + 1) * P, :
                        ].rearrange("b p t -> p b t"),
                        in_=otiles[c][:],
                    )
    nc.finalize()
    return nc


def _get_nc(with_bias: bool) -> bass.Bass:
    if with_bias not in _nc_cache:
        _nc_cache[with_bias] = _build(with_bias)
    return _nc_cache[with_bias]


def kernel(inputs: np.ndarray, w: np.ndarray, b: np.ndarray, **kw) -> np.ndarray:
    x = np.ascontiguousarray(inputs.reshape(B_FULL, T), dtype=np.float32)
    w = np.ascontiguousarray(w, dtype=np.float32)
    b = np.ascontiguousarray(b, dtype=np.float32)
    with_bias = bool(np.any(b))

    nc = _get_nc(with_bias)
    in_maps = []
    for c in range(N_CORES):
        m = {"x": x[c * BS : (c + 1) * BS], "w": w}
        if with_bias:
            m["bvec"] = b
        in_maps.append(m)

    res = run_bass_kernel_spmd(nc, in_maps, core_ids=list(range(N_CORES)))
    out = np.concatenate([r["out"] for r in res.results], axis=0)
    return out


# revision 13
# speedup vs baseline: 1.2519x; 1.2519x over previous
"""Trainium2 Bass kernel for nn_DenseFilterExpansion.

Computes out[b, f, t] = x[b, 0, t] * w[f, t] + bias[f, t] for
x: (128, 1, 4096), w/bias: (256, 4096)  ->  out: (128, 256, 4096) fp32.

Strategy (per core, data-parallel over batch, 16 batches/core):
  - w (and bias, if nonzero) stay resident in SBUF (4 MB each).
  - For each batch b: TensorE broadcasts the row x[b, :] across 128
    partitions via a K=1 ones-matmul into PSUM (nearly free).
  - VectorE multiplies the resident w chunk (128 filters x 4096) by the
    PSUM broadcast, writing (128, bg, 4096) SBUF out tiles.
  - Large contiguous DMAs (HWDGE) store each tile; per partition the
    output row is 16 KB contiguous in DRAM.
The kernel is HBM-write-bound (64 MB of output per core).
"""

import numpy as np

import concourse.bacc as bacc
import concourse.bass as bass
import concourse.mybir as mybir
import concourse.tile as tile
from concourse.bass_utils import run_bass_kernel_spmd

N_CORES = 8
B_FULL = 128
F = 256
T = 4096
BS = B_FULL // N_CORES  # batches per core = 16
P = 128                 # partitions
FP = F // P             # f-chunks = 2
BG = 2                  # batches per output tile
NG = BS // BG           # output tile groups = 8
TH = 2048               # half of T (psum tile width, 4 banks)
MM_N = 512              # matmul free dim (one PSUM bank)

_nc_cache: dict = {}


def _build(with_bias: bool) -> bass.Bass:
    f32 = mybir.dt.float32
    bf16 = mybir.dt.bfloat16
    nc = bacc.Bacc("TRN2", debug=False)

    # x is fed as a 3-way bf16 Dekker split (hi/mid/lo, computed on host):
    # x == hi + mid + lo exactly. The K=3 ones-matmul sums the three rows,
    # reconstructing x bit-exactly in fp32 PSUM while streaming at bf16 rate
    # (the fp32 path costs ~4x: walrus splits fp32 weights HI/LO and fp32
    # moving data streams at half rate).
    x_d = nc.dram_tensor("xs", [BS, 3, T], bf16, kind="ExternalInput")
    w_d = nc.dram_tensor("w", [F, T], f32, kind="ExternalInput")
    b_d = (
        nc.dram_tensor("bvec", [F, T], f32, kind="ExternalInput")
        if with_bias
        else None
    )
    o_d = nc.dram_tensor("out", [BS, F, T], f32, kind="ExternalOutput")

    out_bufs = 2 if with_bias else 3
    with tile.TileContext(nc) as tc:
        with (
            tc.tile_pool(name="const", bufs=1) as cpool,
            tc.tile_pool(name="xstage", bufs=3) as xpool,
            tc.tile_pool(name="outp", bufs=out_bufs) as opool,
            tc.tile_pool(name="psum", bufs=2, space="PSUM") as ppool,
        ):
            ones = cpool.tile([3, P], bf16, tag="ones")
            nc.vector.memset(ones[:], 1.0)

            w_sb = []
            b_sb = []
            for c in range(FP):
                wt = cpool.tile([P, T], f32, tag=f"w{c}")
                nc.sync.dma_start(out=wt[:], in_=w_d[c * P : (c + 1) * P, :])
                w_sb.append(wt)
                if with_bias:
                    bt = cpool.tile([P, T], f32, tag=f"b{c}")
                    nc.sync.dma_start(out=bt[:], in_=b_d[c * P : (c + 1) * P, :])
                    b_sb.append(bt)

            for g in range(NG):
                otiles = [
                    opool.tile([P, BG, T], f32, tag="otile", name=f"ot{g}_{c}")
                    for c in range(FP)
                ]
                for bl in range(BG):
                    bi = g * BG + bl
                    x_row = xpool.tile([3, T], bf16, tag="xrow", name=f"xr{bi}")
                    nc.sync.dma_start(out=x_row[:], in_=x_d[bi, :, :])
                    for h in range(T // TH):
                        ps = ppool.tile([P, TH], f32, tag="ps", name=f"ps{bi}_{h}")
                        for j in range(TH // MM_N):
                            col = h * TH + j * MM_N
                            nc.tensor.matmul(
                                ps[:, j * MM_N : (j + 1) * MM_N],
                                ones[:],
                                x_row[0:3, col : col + MM_N],
                                start=True,
                                stop=True,
                            )
                        for c in range(FP):
                            nc.vector.tensor_mul(
                                out=otiles[c][:, bl, h * TH : (h + 1) * TH],
                                in0=w_sb[c][:, h * TH : (h + 1) * TH],
                                in1=ps[:],
                            )
                            if with_bias:
                                nc.vector.tensor_add(
                                    out=otiles[c][:, bl, h * TH : (h + 1) * TH],
                                    in0=otiles[c][:, bl, h * TH : (h + 1) * TH],
                                    in1=b_sb[c][:, h * TH : (h + 1) * TH],
                                )
                for c in range(FP):
                    nc.sync.dma_start(
                        out=o_d[
                            g * BG : (g + 1) * BG, c * P : (c + 1) * P, :
                        ].rearrange("b p t -> p b t"),
                        in_=otiles[c][:],
                    )
    nc.finalize()
    return nc


def _get_nc(with_bias: bool) -> bass.Bass:
    if with_bias not in _nc_cache:
        _nc_cache[with_bias] = _build(with_bias)
    return _nc_cache[with_bias]


def _split_bf16(x: np.ndarray) -> np.ndarray:
    """Exact 3-way Dekker split: returns (B, 3, T) bf16 with
    hi + mid + lo == x bit-exactly (fp32 sum, normal-range inputs)."""
    import ml_dtypes

    bf = ml_dtypes.bfloat16
    hi = x.astype(bf)
    r1 = x - hi.astype(np.float32)
    mid = r1.astype(bf)
    r2 = r1 - mid.astype(np.float32)
    lo = r2.astype(bf)
    return np.ascontiguousarray(np.stack([hi, mid, lo], axis=1))


def kernel(inputs: np.ndarray, w: np.ndarray, b: np.ndarray, **kw) -> np.ndarray:
    x = np.ascontiguousarray(inputs.reshape(B_FULL, T), dtype=np.float32)
    w = np.ascontiguousarray(w, dtype=np.float32)
    b = np.ascontiguousarray(b, dtype=np.float32)
    with_bias = bool(np.any(b))
    xs = _split_bf16(x)  # (B_FULL, 3, T) bf16

    nc = _get_nc(with_bias)
    in_maps = []
    for c in range(N_CORES):
        m = {"xs": xs[c * BS : (c + 1) * BS], "w": w}
        if with_bias:
            m["bvec"] = b
        in_maps.append(m)

    res = run_bass_kernel_spmd(nc, in_maps, core_ids=list(range(N_CORES)))
    out = np.concatenate([r["out"] for r in res.results], axis=0)
    return out


# revision 14
# speedup vs baseline: 1.4215x; 1.1355x over previous
"""Trainium2 Bass kernel for nn_DenseFilterExpansion.

Computes out[b, f, t] = x[b, 0, t] * w[f, t] + bias[f, t] for
x: (128, 1, 4096), w/bias: (256, 4096)  ->  out: (128, 256, 4096) fp32.

Strategy (per core, data-parallel over batch, 16 batches/core):
  - x is fed as a host-computed 3-way bf16 Dekker split (hi/mid/lo with
    hi+mid+lo == x bit-exactly). A K=3 ones-matmul on TensorE sums the
    three rows while broadcasting across 128 partitions, reconstructing
    x bit-exactly in fp32 PSUM at bf16 streaming rate (the fp32 PE path
    is ~4x slower: HI/LO weight split + half-rate fp32 streaming).
  - w (and bias, if nonzero) stays resident in SBUF.
  - VectorE multiplies the resident w chunk (128 filters x 2048) by the
    PSUM broadcast, writing per-(batch, f-chunk) SBUF out tiles.
  - Each (batch, f-chunk) tile is stored with one 2 MB HWDGE DMA on the
    SP ring (16 KB contiguous per partition); input loads ride the ACT
    ring so they never queue behind output stores.
The kernel is HBM-write-bound (64 MB of output per core, ~360 GB/s).
"""

import numpy as np

import concourse.bacc as bacc
import concourse.bass as bass
import concourse.mybir as mybir
import concourse.tile as tile
from concourse.bass_utils import run_bass_kernel_spmd

N_CORES = 8
B_FULL = 128
F = 256
T = 4096
BS = B_FULL // N_CORES  # batches per core = 16
P = 128                 # partitions
FP = F // P             # f-chunks = 2
TH = 2048               # psum tile width (4 banks)
MM_N = 512              # matmul free dim (one PSUM bank)

_nc_cache: dict = {}


def _build(with_bias: bool) -> bass.Bass:
    f32 = mybir.dt.float32
    bf16 = mybir.dt.bfloat16
    nc = bacc.Bacc("TRN2", debug=False)

    x_d = nc.dram_tensor("xs", [BS, 3, T], bf16, kind="ExternalInput")
    w_d = nc.dram_tensor("w", [F, T], f32, kind="ExternalInput")
    b_d = (
        nc.dram_tensor("bvec", [F, T], f32, kind="ExternalInput")
        if with_bias
        else None
    )
    o_d = nc.dram_tensor("out", [BS, F, T], f32, kind="ExternalOutput")

    out_bufs = 4 if with_bias else 6
    NH = T // TH  # 2 halves
    with tile.TileContext(nc) as tc:
        with (
            tc.tile_pool(name="const", bufs=1) as cpool,
            tc.tile_pool(name="xstage", bufs=4) as xpool,
            tc.tile_pool(name="outp", bufs=out_bufs) as opool,
            tc.tile_pool(name="psum", bufs=2, space="PSUM") as ppool,
        ):
            ones = cpool.tile([3, P], bf16, tag="ones")
            nc.vector.memset(ones[:], 1.0)

            # w (and bias) resident as (c, h) quarter tiles so the first
            # multiply only depends on a 1 MB load.
            w_sb = {}
            b_sb = {}
            for c in range(FP):
                for h in range(NH):
                    wt = cpool.tile([P, TH], f32, tag=f"w{c}_{h}", name=f"w{c}_{h}")
                    nc.scalar.dma_start(
                        out=wt[:], in_=w_d[c * P : (c + 1) * P, h * TH : (h + 1) * TH]
                    )
                    w_sb[c, h] = wt
                    if with_bias:
                        bt = cpool.tile(
                            [P, TH], f32, tag=f"b{c}_{h}", name=f"b{c}_{h}"
                        )
                        nc.scalar.dma_start(
                            out=bt[:],
                            in_=b_d[c * P : (c + 1) * P, h * TH : (h + 1) * TH],
                        )
                        b_sb[c, h] = bt

            for bi in range(BS):
                x_row = xpool.tile([3, T], bf16, tag="xrow", name=f"xr{bi}")
                nc.scalar.dma_start(out=x_row[:], in_=x_d[bi, :, :])
                otiles = [
                    opool.tile([P, T], f32, tag="otile", name=f"ot{bi}_{c}")
                    for c in range(FP)
                ]
                for h in range(NH):
                    ps = ppool.tile([P, TH], f32, tag="ps", name=f"ps{bi}_{h}")
                    for j in range(TH // MM_N):
                        col = h * TH + j * MM_N
                        nc.tensor.matmul(
                            ps[:, j * MM_N : (j + 1) * MM_N],
                            ones[:],
                            x_row[0:3, col : col + MM_N],
                            start=True,
                            stop=True,
                        )
                    for c in range(FP):
                        nc.vector.tensor_mul(
                            out=otiles[c][:, h * TH : (h + 1) * TH],
                            in0=w_sb[c, h][:],
                            in1=ps[:],
                        )
                        if with_bias:
                            nc.vector.tensor_add(
                                out=otiles[c][:, h * TH : (h + 1) * TH],
                                in0=otiles[c][:, h * TH : (h + 1) * TH],
                                in1=b_sb[c, h][:],
                            )
                for c in range(FP):
                    nc.sync.dma_start(
                        out=o_d[bi, c * P : (c + 1) * P, :],
                        in_=otiles[c][:],
                    )
    nc.finalize()
    return nc


def _get_nc(with_bias: bool) -> bass.Bass:
    if with_bias not in _nc_cache:
        _nc_cache[with_bias] = _build(with_bias)
    return _nc_cache[with_bias]


def _split_bf16(x: np.ndarray) -> np.ndarray:
    """Exact 3-way Dekker split: returns (B, 3, T) bf16 with
    hi + mid + lo == x bit-exactly (fp32 sum, normal-range inputs)."""
    import ml_dtypes

    bf = ml_dtypes.bfloat16
    hi = x.astype(bf)
    r1 = x - hi.astype(np.float32)
    mid = r1.astype(bf)
    r2 = r1 - mid.astype(np.float32)
    lo = r2.astype(bf)
    return np.ascontiguousarray(np.stack([hi, mid, lo], axis=1))


def kernel(inputs: np.ndarray, w: np.ndarray, b: np.ndarray, **kw) -> np.ndarray:
    x = np.ascontiguousarray(inputs.reshape(B_FULL, T), dtype=np.float32)
    w = np.ascontiguousarray(w, dtype=np.float32)
    b = np.ascontiguousarray(b, dtype=np.float32)
    with_bias = bool(np.any(b))
    xs = _split_bf16(x)  # (B_FULL, 3, T) bf16

    nc = _get_nc(with_bias)
    in_maps = []
    for c in range(N_CORES):
        m = {"xs": xs[c * BS : (c + 1) * BS], "w": w}
        if with_bias:
            m["bvec"] = b
        in_maps.append(m)

    res = run_bass_kernel_spmd(nc, in_maps, core_ids=list(range(N_CORES)))
    out = np.concatenate([r["out"] for r in res.results], axis=0)
    return out


# revision 16
# speedup vs baseline: 1.4709x; 1.0347x over previous
"""Trainium2 Bass kernel for nn_DenseFilterExpansion.

Computes out[b, f, t] = x[b, 0, t] * w[f, t] + bias[f, t] for
x: (128, 1, 4096), w/bias: (256, 4096)  ->  out: (128, 256, 4096) fp32.

Strategy (per core, data-parallel over batch, 16 batches/core):
  - x is fed as a host-computed 3-way bf16 Dekker split (hi/mid/lo with
    hi+mid+lo == x bit-exactly). A K=3 ones-matmul on TensorE sums the
    three rows while broadcasting across 128 partitions, reconstructing
    x bit-exactly in fp32 PSUM at bf16 streaming rate (the fp32 PE path
    is ~4x slower: HI/LO weight split + half-rate fp32 streaming).
  - w (and bias, if nonzero) stays resident in SBUF.
  - VectorE multiplies the resident w chunk (128 filters x 2048) by the
    PSUM broadcast, writing per-(batch, f-chunk) SBUF out tiles.
  - Each (batch, f-chunk) tile is stored with one 2 MB HWDGE DMA on the
    SP ring (16 KB contiguous per partition); input loads ride the ACT
    ring so they never queue behind output stores.
The kernel is HBM-write-bound (64 MB of output per core, ~360 GB/s).
"""

import numpy as np

import concourse.bacc as bacc
import concourse.bass as bass
import concourse.mybir as mybir
import concourse.tile as tile
from concourse.bass_utils import run_bass_kernel_spmd

N_CORES = 8
B_FULL = 128
F = 256
T = 4096
BS = B_FULL // N_CORES  # batches per core = 16
P = 128                 # partitions
FP = F // P             # f-chunks = 2
TH = 2048               # psum tile width (4 banks)
MM_N = 512              # matmul free dim (one PSUM bank)

_nc_cache: dict = {}


def _build(with_bias: bool) -> bass.Bass:
    f32 = mybir.dt.float32
    bf16 = mybir.dt.bfloat16
    nc = bacc.Bacc("TRN2", debug=False)

    x_d = nc.dram_tensor("xs", [BS, 3, T], bf16, kind="ExternalInput")
    w_d = nc.dram_tensor("w", [F, T], f32, kind="ExternalInput")
    b_d = (
        nc.dram_tensor("bvec", [F, T], f32, kind="ExternalInput")
        if with_bias
        else None
    )
    o_d = nc.dram_tensor("out", [BS, F, T], f32, kind="ExternalOutput")

    out_bufs = 4 if with_bias else 6
    NH = T // TH  # 2 halves
    with tile.TileContext(nc) as tc:
        with (
            tc.tile_pool(name="const", bufs=1) as cpool,
            tc.tile_pool(name="xstage", bufs=4) as xpool,
            tc.tile_pool(name="outp", bufs=out_bufs) as opool,
            tc.tile_pool(name="psum", bufs=2, space="PSUM") as ppool,
        ):
            ones = cpool.tile([3, P], bf16, tag="ones")
            nc.vector.memset(ones[:], 1.0)

            # w (and bias) resident as (c, h) quarter tiles so the first
            # multiply only depends on a 1 MB load.
            w_sb = {}
            b_sb = {}
            for c in range(FP):
                for h in range(NH):
                    wt = cpool.tile([P, TH], f32, tag=f"w{c}_{h}", name=f"w{c}_{h}")
                    nc.scalar.dma_start(
                        out=wt[:], in_=w_d[c * P : (c + 1) * P, h * TH : (h + 1) * TH]
                    )
                    w_sb[c, h] = wt
                    if with_bias:
                        bt = cpool.tile(
                            [P, TH], f32, tag=f"b{c}_{h}", name=f"b{c}_{h}"
                        )
                        nc.scalar.dma_start(
                            out=bt[:],
                            in_=b_d[c * P : (c + 1) * P, h * TH : (h + 1) * TH],
                        )
                        b_sb[c, h] = bt

            for bi in range(BS):
                x_row = xpool.tile([3, T], bf16, tag="xrow", name=f"xr{bi}")
                # SWDGE: separate descriptor path; never queues behind the
                # 2 MB output stores on the two HWDGE rings.
                nc.gpsimd.dma_start(out=x_row[:], in_=x_d[bi, :, :])
                otiles = [
                    opool.tile([P, T], f32, tag="otile", name=f"ot{bi}_{c}")
                    for c in range(FP)
                ]
                for h in range(NH):
                    ps = ppool.tile([P, TH], f32, tag="ps", name=f"ps{bi}_{h}")
                    for j in range(TH // MM_N):
                        col = h * TH + j * MM_N
                        nc.tensor.matmul(
                            ps[:, j * MM_N : (j + 1) * MM_N],
                            ones[:],
                            x_row[0:3, col : col + MM_N],
                            start=True,
                            stop=True,
                        )
                    for c in range(FP):
                        nc.vector.tensor_mul(
                            out=otiles[c][:, h * TH : (h + 1) * TH],
                            in0=w_sb[c, h][:],
                            in1=ps[:],
                        )
                        if with_bias:
                            nc.vector.tensor_add(
                                out=otiles[c][:, h * TH : (h + 1) * TH],
                                in0=otiles[c][:, h * TH : (h + 1) * TH],
                                in1=b_sb[c, h][:],
                            )
                for c in range(FP):
                    # Alternate the two HWDGE rings (SP / ACT) so per-DMA
                    # fixed costs overlap across rings.
                    ring = nc.sync if (bi * FP + c) % 2 == 0 else nc.scalar
                    ring.dma_start(
                        out=o_d[bi, c * P : (c + 1) * P, :],
                        in_=otiles[c][:],
                    )
    nc.finalize()
    return nc


def _get_nc(with_bias: bool) -> bass.Bass:
    if with_bias not in _nc_cache:
        _nc_cache[with_bias] = _build(with_bias)
    return _nc_cache[with_bias]


def _split_bf16(x: np.ndarray) -> np.ndarray:
    """Exact 3-way Dekker split: returns (B, 3, T) bf16 with
    hi + mid + lo == x bit-exactly (fp32 sum, normal-range inputs)."""
    import ml_dtypes

    bf = ml_dtypes.bfloat16
    hi = x.astype(bf)
    r1 = x - hi.astype(np.float32)
    mid = r1.astype(bf)
    r2 = r1 - mid.astype(np.float32)
    lo = r2.astype(bf)
    return np.ascontiguousarray(np.stack([hi, mid, lo], axis=1))


def kernel(inputs: np.ndarray, w: np.ndarray, b: np.ndarray, **kw) -> np.ndarray:
    x = np.ascontiguousarray(inputs.reshape(B_FULL, T), dtype=np.float32)
    w = np.ascontiguousarray(w, dtype=np.float32)
    b = np.ascontiguousarray(b, dtype=np.float32)
    with_bias = bool(np.any(b))
    xs = _split_bf16(x)  # (B_FULL, 3, T) bf16

    nc = _get_nc(with_bias)
    in_maps = []
    for c in range(N_CORES):
        m = {"xs": xs[c * BS : (c + 1) * BS], "w": w}
        if with_bias:
            m["bvec"] = b
        in_maps.append(m)

    res = run_bass_kernel_spmd(nc, in_maps, core_ids=list(range(N_CORES)))
    out = np.concatenate([r["out"] for r in res.results], axis=0)
    return out


# revision 18
# speedup vs baseline: 1.5001x; 1.0199x over previous
"""Trainium2 Bass kernel for nn_DenseFilterExpansion.

Computes out[b, f, t] = x[b, 0, t] * w[f, t] + bias[f, t] for
x: (128, 1, 4096), w/bias: (256, 4096)  ->  out: (128, 256, 4096) fp32.

Strategy (per core, data-parallel over batch, 16 batches/core):
  - x is fed as a host-computed 3-way bf16 Dekker split (hi/mid/lo with
    hi+mid+lo == x bit-exactly). A K=3 ones-matmul on TensorE sums the
    three rows while broadcasting across 128 partitions, reconstructing
    x bit-exactly in fp32 PSUM at bf16 streaming rate (the fp32 PE path
    is ~4x slower: HI/LO weight split + half-rate fp32 streaming).
  - w (and bias, if nonzero) stays resident in SBUF.
  - VectorE multiplies the resident w chunk (128 filters x 2048) by the
    PSUM broadcast, writing per-(batch, f-chunk) SBUF out tiles.
  - Each (batch, f-chunk) tile is stored with one 2 MB HWDGE DMA on the
    SP ring (16 KB contiguous per partition); input loads ride the ACT
    ring so they never queue behind output stores.
The kernel is HBM-write-bound (64 MB of output per core, ~360 GB/s).
"""

import numpy as np

import concourse.bacc as bacc
import concourse.bass as bass
import concourse.mybir as mybir
import concourse.tile as tile
from concourse.bass_utils import run_bass_kernel_spmd

N_CORES = 8
B_FULL = 128
F = 256
T = 4096
BS = B_FULL // N_CORES  # batches per core = 16
P = 128                 # partitions
FP = F // P             # f-chunks = 2
TH = 2048               # psum tile width (4 banks)
MM_N = 512              # matmul free dim (one PSUM bank)

_nc_cache: dict = {}


def _build(with_bias: bool) -> bass.Bass:
    f32 = mybir.dt.float32
    bf16 = mybir.dt.bfloat16
    nc = bacc.Bacc("TRN2", debug=False)

    x_d = nc.dram_tensor("xs", [BS, 3, T], bf16, kind="ExternalInput")
    w_d = nc.dram_tensor("w", [F, T], f32, kind="ExternalInput")
    b_d = (
        nc.dram_tensor("bvec", [F, T], f32, kind="ExternalInput")
        if with_bias
        else None
    )
    o_d = nc.dram_tensor("out", [BS, F, T], f32, kind="ExternalOutput")

    out_bufs = 4 if with_bias else 6
    NH = T // TH  # 2 halves
    with tile.TileContext(nc) as tc:
        with (
            tc.tile_pool(name="const", bufs=1) as cpool,
            tc.tile_pool(name="xstage", bufs=4) as xpool,
            tc.tile_pool(name="outp", bufs=out_bufs) as opool,
            tc.tile_pool(name="psum", bufs=2, space="PSUM") as ppool,
        ):
            ones = cpool.tile([3, P], bf16, tag="ones")
            nc.vector.memset(ones[:], 1.0)

            # Prefetch the first x rows before anything else so the PE can
            # start broadcasting as soon as the preamble finishes.
            x_rows = {}
            for bi in range(min(BS, 4)):
                xr = xpool.tile([3, T], bf16, tag="xrow", name=f"xr{bi}")
                nc.gpsimd.dma_start(out=xr[:], in_=x_d[bi, :, :])
                x_rows[bi] = xr

            # w (and bias) resident as (c, h) quarter tiles so the first
            # multiply only depends on a 1 MB load.
            w_sb = {}
            b_sb = {}
            for c in range(FP):
                for h in range(NH):
                    wt = cpool.tile([P, TH], f32, tag=f"w{c}_{h}", name=f"w{c}_{h}")
                    nc.scalar.dma_start(
                        out=wt[:], in_=w_d[c * P : (c + 1) * P, h * TH : (h + 1) * TH]
                    )
                    w_sb[c, h] = wt
                    if with_bias:
                        bt = cpool.tile(
                            [P, TH], f32, tag=f"b{c}_{h}", name=f"b{c}_{h}"
                        )
                        nc.scalar.dma_start(
                            out=bt[:],
                            in_=b_d[c * P : (c + 1) * P, h * TH : (h + 1) * TH],
                        )
                        b_sb[c, h] = bt

            for bi in range(BS):
                if bi in x_rows:
                    x_row = x_rows[bi]
                else:
                    x_row = xpool.tile([3, T], bf16, tag="xrow", name=f"xr{bi}")
                    # SWDGE: separate descriptor path; never queues behind
                    # the 2 MB output stores on the two HWDGE rings.
                    nc.gpsimd.dma_start(out=x_row[:], in_=x_d[bi, :, :])
                otiles = [
                    opool.tile([P, T], f32, tag="otile", name=f"ot{bi}_{c}")
                    for c in range(FP)
                ]
                for h in range(NH):
                    ps = ppool.tile([P, TH], f32, tag="ps", name=f"ps{bi}_{h}")
                    for j in range(TH // MM_N):
                        col = h * TH + j * MM_N
                        nc.tensor.matmul(
                            ps[:, j * MM_N : (j + 1) * MM_N],
                            ones[:],
                            x_row[0:3, col : col + MM_N],
                            start=True,
                            stop=True,
                        )
                    for c in range(FP):
                        nc.vector.tensor_mul(
                            out=otiles[c][:, h * TH : (h + 1) * TH],
                            in0=w_sb[c, h][:],
                            in1=ps[:],
                        )
                        if with_bias:
                            nc.vector.tensor_add(
                                out=otiles[c][:, h * TH : (h + 1) * TH],
                                in0=otiles[c][:, h * TH : (h + 1) * TH],
                                in1=b_sb[c, h][:],
                            )
                for c in range(FP):
                    # Alternate the two HWDGE rings (SP / ACT) so per-DMA
                    # fixed costs overlap across rings.
                    ring = nc.sync if (bi * FP + c) % 2 == 0 else nc.scalar
                    ring.dma_start(
                        out=o_d[bi, c * P : (c + 1) * P, :],
                        in_=otiles[c][:],
                    )
    nc.finalize()
    return nc


def _get_nc(with_bias: bool) -> bass.Bass:
    if with_bias not in _nc_cache:
        _nc_cache[with_bias] = _build(with_bias)
    return _nc_cache[with_bias]


def _split_bf16(x: np.ndarray) -> np.ndarray:
    """Exact 3-way Dekker split: returns (B, 3, T) bf16 with
    hi + mid + lo == x bit-exactly (fp32 sum, normal-range inputs)."""
    import ml_dtypes

    bf = ml_dtypes.bfloat16
    hi = x.astype(bf)
    r1 = x - hi.astype(np.float32)
    mid = r1.astype(bf)
    r2 = r1 - mid.astype(np.float32)
    lo = r2.astype(bf)
    return np.ascontiguousarray(np.stack([hi, mid, lo], axis=1))


def kernel(inputs: np.ndarray, w: np.ndarray, b: np.ndarray, **kw) -> np.ndarray:
    x = np.ascontiguousarray(inputs.reshape(B_FULL, T), dtype=np.float32)
    w = np.ascontiguousarray(w, dtype=np.float32)
    b = np.ascontiguousarray(b, dtype=np.float32)
    with_bias = bool(np.any(b))
    xs = _split_bf16(x)  # (B_FULL, 3, T) bf16

    nc = _get_nc(with_bias)
    in_maps = []
    for c in range(N_CORES):
        m = {"xs": xs[c * BS : (c + 1) * BS], "w": w}
        if with_bias:
            m["bvec"] = b
        in_maps.append(m)

    res = run_bass_kernel_spmd(nc, in_maps, core_ids=list(range(N_CORES)))
    out = np.concatenate([r["out"] for r in res.results], axis=0)
    return out
